# revision 1
# baseline (speedup 1.0000x reference)
"""Distributed GQA attention kernel for 8 TRN2 NeuronCores.

Problem: B=2, S=2048, D=1280, NH=16 q-heads, NKV=8 kv-heads, HD=80 (RoPE +
per-head QK RMSNorm, causal, GQA rep=2, o_proj).

Sharding: data-parallel over batch (cores 0-3 -> batch 0, cores 4-7 ->
batch 1) x tensor-parallel over kv-head groups (4 groups per batch; each
core owns 4 q heads + 2 kv heads).  Each core computes a partial o_proj
(row-shard of Wo); the host sums the 4 partials per batch (the "all-reduce")
and stacks the two batches.

Device-side design (per core, all matmul operands bf16, f32 accumulation):
  - host pre-transposes x -> xT so the contraction dim (D) is on partitions
  - dense natural-layout QKV projection: psum[s,480qk]/[s,160v]
  - RMS stats via ACT square + DVE grouped reduce; RoPE via free-axis
    tensor ops with host-precomputed tables (norm weights folded into the
    tables; the rms scale is applied after RoPE - it commutes).
  - q/k transposed per head via PE transpose -> qT/kT [80, S] layouts
  - scores^T [kv, q] = K @ Q^T per (head, q-tile 512, kv-tile 128); exp with
    fused 1/sqrt(HD) scale on ScalarE straight out of PSUM (no max
    subtraction needed: RMS-normed rows bound |scores/sqrt(HD)| <= sqrt(80))
  - causal handled by multiplying the diagonal-band region with a
    precomputed 0/1 mask after exp
  - PV with V augmented by a ones column (at 32-aligned partition 96) ->
    softmax denominator rides along in the same accumulation
  - normalize via DVE reciprocal + a tiny ones-matmul broadcast that reuses
    the pv psum bank (bf16 operands: fp32 matmul is 4x slower)
  - o_proj accumulates the 4 heads into psum, copied to SBUF and DMA'd out;
    the host sums the 4 partial outputs per batch
"""

import numpy as np
import ml_dtypes

B, S, D = 2, 2048, 1280
NH, NKV, HD = 16, 8, 80
REP = NH // NKV
EPS = 1e-6
THETA = 1e6
NCORES = 8
GROUPS = 4          # TP groups per batch
QH = NH // GROUPS   # 4 q heads per core
KH = NKV // GROUPS  # 2 kv heads per core
NT = S // 128       # 16 s-tiles
NJ = S // 512       # 4 q-tiles
SCALE = 1.0 / float(np.sqrt(HD))

BF16 = ml_dtypes.bfloat16


def _build_host_consts(Wq, Wk, Wv, Wo, q_norm_w, k_norm_w):
    """Per-TP-group weight shards + shared tables."""
    inv_freq = 1.0 / (THETA ** (np.arange(0, HD, 2, dtype=np.float64) / HD))
    t = np.arange(S, dtype=np.float64)
    freqs = np.outer(t, inv_freq)          # [S, 40]
    cos = np.cos(freqs).astype(np.float32)
    sin = np.sin(freqs).astype(np.float32)

    def rope_block(w):
        # [S, 160] = [C1|S1|C2|S2]; norm weight w folded in.
        c1 = cos * w[None, :40]
        s1 = sin * w[None, 40:]
        c2 = cos * w[None, 40:]
        s2 = sin * w[None, :40]
        return np.concatenate([c1, s1, c2, s2], axis=1)  # [S, 160]

    rq = np.tile(rope_block(q_norm_w), (1, QH)).astype(BF16)   # [S, 640]
    rk = np.tile(rope_block(k_norm_w), (1, KH)).astype(BF16)   # [S, 320]

    # band mask [128, 4, 512]: mb[p, r, c] = 1 if 128r + p <= c
    p = np.arange(128)[:, None, None]
    r = np.arange(4)[None, :, None]
    c = np.arange(512)[None, None, :]
    mb = ((128 * r + p) <= c).astype(BF16)                      # [128,4,512]

    ident = np.eye(128, dtype=BF16)

    shards = []
    for g in range(GROUPS):
        wq = Wq[:, g * QH * HD:(g + 1) * QH * HD]               # [D, 320]
        wk = Wk[:, g * KH * HD:(g + 1) * KH * HD]               # [D, 160]
        wv = Wv[:, g * KH * HD:(g + 1) * KH * HD]               # [D, 160]
        wqkv = np.concatenate([wq, wk, wv], axis=1).astype(BF16)  # [D, 640]
        wo_g = Wo[g * QH * HD:(g + 1) * QH * HD, :]             # [320, D]
        # packed for 3-matmul o_proj: A = [h0 d0:64 | h1 d0:64],
        # B = [h2 d0:64 | h3 d0:64], C = 32-blocks [h d64:80 | zeros16]
        woA = np.concatenate([wo_g[0:64], wo_g[80:144]], axis=0)
        woB = np.concatenate([wo_g[160:224], wo_g[240:304]], axis=0)
        woC = np.zeros((128, D), np.float32)
        for h in range(QH):
            woC[32 * h:32 * h + 16] = wo_g[80 * h + 64:80 * h + 80]
        wo = np.stack([woA, woB, woC]).transpose(1, 0, 2).reshape(
            128, 3 * D).astype(BF16)                            # [128, 3*1280]
        shards.append((wqkv, wo))
    return rq, rk, mb, ident, shards


def _build_graph():
    import concourse.bacc as bacc
    import concourse.mybir as mybir
    from concourse.tile import TileContext
    from concourse import bass_isa

    f32 = mybir.dt.float32
    bf16 = mybir.dt.bfloat16
    AF = mybir.ActivationFunctionType

    nc = bacc.Bacc("TRN2", target_bir_lowering=False, debug=False,
                   num_devices=NCORES)

    xT_d = nc.dram_tensor("xT", [10, 128, S], bf16, kind="ExternalInput")
    wqkv_d = nc.dram_tensor("wqkv", [10, 128, 640], bf16, kind="ExternalInput")
    wo_d = nc.dram_tensor("wo", [128, 3 * D], bf16, kind="ExternalInput")
    rq_d = nc.dram_tensor("ropeq", [128, NT * 640], bf16, kind="ExternalInput")
    rk_d = nc.dram_tensor("ropek", [128, NT * 320], bf16, kind="ExternalInput")
    mb_d = nc.dram_tensor("maskband", [128, 2048], bf16, kind="ExternalInput")
    id_d = nc.dram_tensor("ident", [128, 128], bf16, kind="ExternalInput")
    out_d = nc.dram_tensor("out", [S, D], f32, kind="ExternalOutput")

    with TileContext(nc) as tc:
        with (
            tc.tile_pool(name="const", bufs=1) as cp,
            tc.tile_pool(name="persist", bufs=1) as pp,
        ):
            xT_sb = cp.tile([128, 10, S], bf16)
            w_sb = cp.tile([128, 10, 640], bf16)
            wo_sb = cp.tile([128, 3, D], bf16)
            rq_sb = cp.tile([128, NT, 640], bf16)
            rk_sb = cp.tile([128, NT, 320], bf16)
            mb_sb = cp.tile([128, 4, 512], bf16)
            id_sb = cp.tile([128, 128], bf16)
            eps_sb = cp.tile([128, 1], f32)

            qT_sb = pp.tile([128, QH, S], bf16)
            kT_sb = pp.tile([128, KH, S], bf16)
            v_sb = pp.tile([128, NT, KH, 97], bf16)
            rbz_sb = pp.tile([96, 512], f32)

            for c in range(10):
                nc.sync.dma_start(out=w_sb[:, c, :], in_=wqkv_d[c])
                nc.sync.dma_start(out=xT_sb[:, c, :], in_=xT_d[c])
            nc.sync.dma_start(out=wo_sb[:], in_=wo_d[:])
            nc.sync.dma_start(out=rq_sb[:], in_=rq_d[:])
            nc.sync.dma_start(out=rk_sb[:], in_=rk_d[:])
            nc.sync.dma_start(out=mb_sb[:], in_=mb_d[:])
            nc.sync.dma_start(out=id_sb[:], in_=id_d[:])
            nc.vector.memset(eps_sb[:], EPS)
            nc.vector.memset(rbz_sb[:], 0.0)
            nc.vector.memset(v_sb[:, :, :, HD:97], 0.0)
            nc.vector.memset(v_sb[:, :, :, 96:97], 1.0)

            # ---------------- phase 1: QKV projection + norm + rope ----------
            with (
                tc.tile_pool(name="p1ps", bufs=2, space="PSUM") as ps1,
                tc.tile_pool(name="p1w", bufs=6) as wp,
            ):
                for t in range(NT):
                    qk_ps = ps1.tile([128, 480], f32, tag="qk", bufs=3)
                    v_ps = ps1.tile([128, 160], f32, tag="v", bufs=2)
                    for c in range(10):
                        lhs = xT_sb[:, c, 128 * t:128 * (t + 1)]
                        nc.tensor.matmul(qk_ps[:], lhs, w_sb[:, c, 0:480],
                                         start=(c == 0), stop=(c == 9))
                        nc.tensor.matmul(v_ps[:], lhs, w_sb[:, c, 480:640],
                                         start=(c == 0), stop=(c == 9))
                    # v -> augmented V (ones col pre-set)
                    nc.scalar.copy(
                        v_sb[:, t, :, 0:HD],
                        v_ps.rearrange("p (g d) -> p g d", g=KH),
                    )
                    # rms stats
                    qk_nat = wp.tile([128, 480], bf16, tag="qknat")
                    nc.scalar.copy(qk_nat[:], qk_ps[:])
                    sq = wp.tile([128, 480], f32, tag="sq")
                    nc.scalar.activation(sq[:], qk_ps[:], AF.Square)
                    ssum = wp.tile([128, 6], f32, tag="ssum")
                    nc.vector.tensor_reduce(
                        ssum[:], sq.rearrange("p (h d) -> p h d", d=HD),
                        axis=mybir.AxisListType.X, op=mybir.AluOpType.add)
                    rmsq = wp.tile([128, 6], f32, tag="rmsq")
                    nc.scalar.activation(rmsq[:], ssum[:], AF.Sqrt,
                                         scale=1.0 / HD, bias=eps_sb[:])
                    rms = wp.tile([128, 6], f32, tag="rms")
                    nc.vector.reciprocal(rms[:], rmsq[:])
                    last_rmsq = rmsq

                    # rope (tables carry the norm weights)
                    qk3 = qk_nat.rearrange("p (h d) -> p h d", d=HD)
                    rope = wp.tile([128, 480], bf16, tag="rope")
                    ro3 = rope.rearrange("p (h d) -> p h d", d=HD)
                    tq1 = wp.tile([128, QH, 40], bf16, tag="tq1")
                    tq2 = wp.tile([128, QH, 40], bf16, tag="tq2")
                    tk1 = wp.tile([128, KH, 40], bf16, tag="tk1")
                    tk2 = wp.tile([128, KH, 40], bf16, tag="tk2")
                    rqt = rq_sb[:, t, :].rearrange("p (h four d) -> p h four d",
                                                   four=4, d=40)
                    rkt = rk_sb[:, t, :].rearrange("p (h four d) -> p h four d",
                                                   four=4, d=40)
                    # q halves
                    nc.vector.tensor_mul(tq1[:], qk3[:, 0:QH, 0:40], rqt[:, :, 0, :])
                    nc.vector.tensor_mul(tq2[:], qk3[:, 0:QH, 40:HD], rqt[:, :, 1, :])
                    nc.vector.tensor_sub(ro3[:, 0:QH, 0:40], tq1[:], tq2[:])
                    nc.vector.tensor_mul(tq1[:], qk3[:, 0:QH, 40:HD], rqt[:, :, 2, :])
                    nc.vector.tensor_mul(tq2[:], qk3[:, 0:QH, 0:40], rqt[:, :, 3, :])
                    nc.vector.tensor_add(ro3[:, 0:QH, 40:HD], tq1[:], tq2[:])
                    # k halves on gpsimd (frees DVE, the phase-1 bottleneck)
                    kof = QH
                    nc.gpsimd.tensor_mul(tk1[:], qk3[:, kof:kof + KH, 0:40], rkt[:, :, 0, :])
                    nc.gpsimd.tensor_mul(tk2[:], qk3[:, kof:kof + KH, 40:HD], rkt[:, :, 1, :])
                    nc.gpsimd.tensor_sub(ro3[:, kof:kof + KH, 0:40], tk1[:], tk2[:])
                    nc.gpsimd.tensor_mul(tk1[:], qk3[:, kof:kof + KH, 40:HD], rkt[:, :, 2, :])
                    nc.gpsimd.tensor_mul(tk2[:], qk3[:, kof:kof + KH, 0:40], rkt[:, :, 3, :])
                    nc.gpsimd.tensor_add(ro3[:, kof:kof + KH, 40:HD], tk1[:], tk2[:])

                    # apply rms scale per head -> qn
                    qn = wp.tile([128, 6, 128], bf16, tag="qn")
                    qn3 = qn[:, :, 0:HD]
                    for h in range(6):
                        eng = nc.vector if h < QH else nc.gpsimd
                        eng.tensor_scalar_mul(qn3[:, h, :], ro3[:, h, :],
                                              rms[:, h:h + 1])
                    # transpose each head -> qT/kT (PE transpose, copies split
                    # between ACT and DVE)
                    for h in range(6):
                        tp = ps1.tile([HD, 128], bf16, tag="tp", bufs=3)
                        nc.tensor.transpose(tp[:], qn3[:, h, :], id_sb[:])
                        if h < QH:
                            dest = qT_sb[0:HD, h, 128 * t:128 * (t + 1)]
                        else:
                            dest = kT_sb[0:HD, h - QH, 128 * t:128 * (t + 1)]
                        if h % 2 == 0:
                            nc.vector.tensor_copy(dest, tp[:])
                        else:
                            nc.scalar.copy(dest, tp[:])

                # pre-load the exp ACT table right after the LAST Sqrt: the
                # data dependency on last_rmsq stops the scheduler from
                # hoisting it before the tail Sqrts (which would thrash the
                # table), and the remaining phase-1 ACT ops are set-agnostic
                # Copies - so the exp set is resident before the first scores
                warm = wp.tile([128, 1], f32, tag="warm")
                nc.scalar.activation(warm[:], last_rmsq[:, 0:1],
                                     AF.Exp)

            # ---------------- phase 2: attention + o_proj --------------------
            with (
                tc.tile_pool(name="ppv", bufs=3, space="PSUM") as ppv,
                tc.tile_pool(name="pop", bufs=1, space="PSUM") as pop,
                tc.tile_pool(name="psc", bufs=2, space="PSUM") as psc,
                tc.tile_pool(name="slabp", bufs=3) as slabp,
                tc.tile_pool(name="attnp", bufs=2) as attnp,
                tc.tile_pool(name="smallp", bufs=3) as smallp,
            ):
                for j in range(NJ):
                    atA = attnp.tile([128, 512], bf16, tag="attnA")
                    atB = attnp.tile([128, 512], bf16, tag="attnB")
                    atC = attnp.tile([128, 512], bf16, tag="attnC")
                    for h in range(QH):
                        g2 = h // REP
                        ntiles = 4 * (j + 1)
                        slab = slabp.tile([128, NT, 512], bf16, tag="slab")
                        for i2 in range(0, ntiles, 2):
                            sc = psc.tile([128, 1024], f32, tag="sc")
                            for ii in range(2):
                                i = i2 + ii
                                r = i - 4 * j
                                c0 = 128 * r if r > 0 else 0
                                nc.tensor.matmul(
                                    sc[:, 512 * ii + c0:512 * (ii + 1)],
                                    kT_sb[0:HD, g2, 128 * i:128 * (i + 1)],
                                    qT_sb[0:HD, h, 512 * j + c0:512 * (j + 1)],
                                    start=True, stop=True)
                            rlo = i2 - 4 * j
                            e0 = 128 * rlo if rlo > 0 else 0
                            nc.scalar.activation(
                                slab[:, i2:i2 + 2, e0:512],
                                sc.rearrange("p (a b) -> p a b", a=2)[:, :, e0:512],
                                AF.Exp, scale=SCALE)
                        # causal diagonal-block masks (cols below the
                        # block are skipped by the narrowed PV reads)
                        for r in range(4):
                            i = 4 * j + r
                            nc.vector.tensor_mul(
                                slab[:, i, 128 * r:128 * (r + 1)],
                                slab[:, i, 128 * r:128 * (r + 1)],
                                mb_sb[:, r, 128 * r:128 * (r + 1)])
                        # PV with ones column -> denominator on row HD
                        pv = ppv.tile([97, 512], f32, tag="pv")
                        for i in range(ntiles):
                            r = i - 4 * j
                            c0 = 128 * r if r > 0 else 0
                            nc.tensor.matmul(pv[:, c0:512], v_sb[:, i, g2, :],
                                             slab[:, i, c0:512],
                                             start=(i == 0),
                                             stop=(i == ntiles - 1))
                        # reciprocal of the denominator row into partition 0
                        # of a zero-padded staging tile, then broadcast to all
                        # 80 partitions via gpsimd partition_all_reduce (sums
                        # the single nonzero row)
                        nc.vector.reciprocal(rbz_sb[0:1, :], pv[96:97, :])
                        rb = smallp.tile([96, 512], f32, tag="rb", bufs=4)
                        nc.gpsimd.partition_all_reduce(
                            rb[:], rbz_sb[:], channels=96,
                            reduce_op=bass_isa.ReduceOp.add)
                        # write the packed o_proj operand tiles: d0:64 into
                        # A/B halves, d64:80 (+16 clean-zero rows from the V
                        # padding) into C's 32-blocks
                        ab = atA if h < 2 else atB
                        p0 = 64 * (h % 2)
                        nc.vector.tensor_mul(ab[p0:p0 + 64, :], pv[0:64, :],
                                             rb[0:64, :])
                        nc.vector.tensor_mul(atC[32 * h:32 * h + 32, :],
                                             pv[64:96, :], rb[64:96, :])
                    # o_proj for this q-tile: accumulate the 4 heads
                    for st in range(4):
                        row0 = 512 * j + 128 * st
                        for nsl_i, (n0, nw) in enumerate(((0, 512), (512, 512), (1024, 256))):
                            if j == NJ - 1:
                                # last q-tile: nothing follows, so borrow the
                                # idle scores/pv banks for a deeper burst
                                k3 = (3 * st + nsl_i) % 3
                                if k3 == 0:
                                    op = pop.tile([128, nw], f32, tag="op")
                                elif k3 == 1:
                                    op = ppv.tile([128, nw], f32, tag="pv")
                                else:
                                    op = psc.tile([128, nw], f32, tag="sc")
                            else:
                                op = pop.tile([128, nw], f32, tag="op")
                            for ki, at in enumerate((atA, atB, atC)):
                                nc.tensor.matmul(
                                    op[:],
                                    at[:, 128 * st:128 * (st + 1)],
                                    wo_sb[:, ki, n0:n0 + nw],
                                    start=(ki == 0), stop=(ki == 2))
                            ob = smallp.tile([128, nw], f32, tag="ob", bufs=6)
                            nc.vector.tensor_copy(ob[:], op[:])
                            nc.sync.dma_start(
                                out=out_d[row0:row0 + 128, n0:n0 + nw],
                                in_=ob[:])
    return nc


_GRAPH_CACHE = {}


def _get_graph():
    if "nc" not in _GRAPH_CACHE:
        nc = _build_graph()
        nc.finalize()
        _GRAPH_CACHE["nc"] = nc
    return _GRAPH_CACHE["nc"]


def kernel(x, Wq, Wk, Wv, Wo, q_norm_w, k_norm_w, _trace=False):
    from concourse.bass_utils import run_bass_kernel_spmd

    x = np.asarray(x, dtype=np.float32)
    Wq = np.asarray(Wq, dtype=np.float32)
    Wk = np.asarray(Wk, dtype=np.float32)
    Wv = np.asarray(Wv, dtype=np.float32)
    Wo = np.asarray(Wo, dtype=np.float32)
    q_norm_w = np.asarray(q_norm_w, dtype=np.float32)
    k_norm_w = np.asarray(k_norm_w, dtype=np.float32)

    rq, rk, mb, ident, shards = _build_host_consts(Wq, Wk, Wv, Wo,
                                                   q_norm_w, k_norm_w)
    # partition-major: row p holds [t, f] blocks so the DMA is 2D contiguous
    rq = np.ascontiguousarray(
        rq.reshape(NT, 128, 640).transpose(1, 0, 2).reshape(128, NT * 640))
    rk = np.ascontiguousarray(
        rk.reshape(NT, 128, 320).transpose(1, 0, 2).reshape(128, NT * 320))

    in_maps = []
    for core in range(NCORES):
        b = core // GROUPS
        g = core % GROUPS
        wqkv, wo = shards[g]
        xT = np.ascontiguousarray(x[b].T.astype(BF16)).reshape(10, 128, S)
        in_maps.append({
            "xT": xT,
            "wqkv": np.ascontiguousarray(wqkv.reshape(10, 128, 640)),
            "wo": wo,
            "ropeq": rq,
            "ropek": rk,
            "maskband": np.ascontiguousarray(mb.reshape(128, 2048)),
            "ident": ident,
        })

    nc = _get_graph()
    res = run_bass_kernel_spmd(nc, in_maps, core_ids=list(range(NCORES)),
                               trace=_trace)
    outs = [r["out"] for r in res.results]
    full = np.zeros((B, S, D), dtype=np.float32)
    for core in range(NCORES):
        full[core // GROUPS] += outs[core]
    if _trace:
        kernel.last_results = res
    return full



# revision 2
# speedup vs baseline: 1.0134x; 1.0134x over previous
"""Distributed GQA attention kernel for 8 TRN2 NeuronCores.

Problem: B=2, S=2048, D=1280, NH=16 q-heads, NKV=8 kv-heads, HD=80 (RoPE +
per-head QK RMSNorm, causal, GQA rep=2, o_proj).

Sharding: data-parallel over batch (cores 0-3 -> batch 0, cores 4-7 ->
batch 1) x tensor-parallel over kv-head groups (4 groups per batch; each
core owns 4 q heads + 2 kv heads).  Each core computes a partial o_proj
(row-shard of Wo); the host sums the 4 partials per batch (the "all-reduce")
and stacks the two batches.

Two-phase device design (per core, bf16 matmuls, f32 accumulation), as the
proven baseline, plus three scheduling fixes:
  - input DMAs split per 512-row block (first block halved) so the first
    QKV matmuls start at ~5us instead of ~12us
  - phase 2 software-pipelines PE work against the ACT exp drain: PV of the
    previous head and o_proj chunks of the previous q-block are emitted
    between score pairs, so the PE has filler work while ACT chews exp
  - output is written bf16 (halves the output DMA; host sums partials f32)
"""

import numpy as np
import ml_dtypes

B, S, D = 2, 2048, 1280
NH, NKV, HD = 16, 8, 80
REP = NH // NKV
EPS = 1e-6
THETA = 1e6
NCORES = 8
GROUPS = 4          # TP groups per batch
QH = NH // GROUPS   # 4 q heads per core
KH = NKV // GROUPS  # 2 kv heads per core
NT = S // 128       # 16 s-tiles
NJ = S // 512       # 4 q-tiles
SCALE = 1.0 / float(np.sqrt(HD))

BF16 = ml_dtypes.bfloat16


def _build_host_consts(Wq, Wk, Wv, Wo, q_norm_w, k_norm_w):
    """Per-TP-group weight shards + shared tables."""
    inv_freq = 1.0 / (THETA ** (np.arange(0, HD, 2, dtype=np.float64) / HD))
    t = np.arange(S, dtype=np.float64)
    freqs = np.outer(t, inv_freq)          # [S, 40]
    cos = np.cos(freqs).astype(np.float32)
    sin = np.sin(freqs).astype(np.float32)

    def rope_block(w):
        # [S, 160] = [C1|S1|C2|S2]; norm weight w folded in.
        c1 = cos * w[None, :40]
        s1 = sin * w[None, 40:]
        c2 = cos * w[None, 40:]
        s2 = sin * w[None, :40]
        return np.concatenate([c1, s1, c2, s2], axis=1)  # [S, 160]

    rq = np.tile(rope_block(q_norm_w), (1, QH)).astype(BF16)   # [S, 640]
    rk = np.tile(rope_block(k_norm_w), (1, KH)).astype(BF16)   # [S, 320]
    # partition-major: row p holds [t, f] blocks
    rq = np.ascontiguousarray(
        rq.reshape(NT, 128, QH * 160).transpose(1, 0, 2).reshape(128, NT * 640))
    rk = np.ascontiguousarray(
        rk.reshape(NT, 128, KH * 160).transpose(1, 0, 2).reshape(128, NT * 320))

    # band mask [128, 4, 512]: mb[p, r, c] = 1 if 128r + p <= c
    p = np.arange(128)[:, None, None]
    r = np.arange(4)[None, :, None]
    c = np.arange(512)[None, None, :]
    mb = ((128 * r + p) <= c).astype(BF16)                      # [128,4,512]
    mb = np.ascontiguousarray(mb.reshape(128, 2048))

    ident = np.eye(128, dtype=BF16)

    shards = []
    for g in range(GROUPS):
        wq = Wq[:, g * QH * HD:(g + 1) * QH * HD]               # [D, 320]
        wk = Wk[:, g * KH * HD:(g + 1) * KH * HD]               # [D, 160]
        wv = Wv[:, g * KH * HD:(g + 1) * KH * HD]               # [D, 160]
        wqkv = np.concatenate([wq, wk, wv], axis=1).astype(BF16)  # [D, 640]
        # p-major [128, 10, 640]
        wqkv = np.ascontiguousarray(
            wqkv.reshape(10, 128, 640).transpose(1, 0, 2))
        wo_g = Wo[g * QH * HD:(g + 1) * QH * HD, :]             # [320, D]
        # packed for 3-matmul o_proj: A = [h0 d0:64 | h1 d0:64],
        # B = [h2 d0:64 | h3 d0:64], C = 32-blocks [h d64:80 | zeros16]
        woA = np.concatenate([wo_g[0:64], wo_g[80:144]], axis=0)
        woB = np.concatenate([wo_g[160:224], wo_g[240:304]], axis=0)
        woC = np.zeros((128, D), np.float32)
        for h in range(QH):
            woC[32 * h:32 * h + 16] = wo_g[80 * h + 64:80 * h + 80]
        wo = np.stack([woA, woB, woC]).transpose(1, 0, 2).reshape(
            128, 3 * D).astype(BF16)                            # [128, 3*1280]
        shards.append((wqkv, wo))
    return rq, rk, mb, ident, shards


def _build_graph():
    import concourse.bacc as bacc
    import concourse.mybir as mybir
    from concourse.tile import TileContext
    from concourse import bass_isa

    f32 = mybir.dt.float32
    bf16 = mybir.dt.bfloat16
    AF = mybir.ActivationFunctionType

    nc = bacc.Bacc("TRN2", target_bir_lowering=False, debug=False,
                   num_devices=NCORES)

    xT_d = nc.dram_tensor("xT", [NJ, 128, 10, 512], bf16, kind="ExternalInput")
    wqkv_d = nc.dram_tensor("wqkv", [128, 10, 640], bf16, kind="ExternalInput")
    wo_d = nc.dram_tensor("wo", [128, 3 * D], bf16, kind="ExternalInput")
    rq_d = nc.dram_tensor("ropeq", [128, NT * 640], bf16, kind="ExternalInput")
    rk_d = nc.dram_tensor("ropek", [128, NT * 320], bf16, kind="ExternalInput")
    mb_d = nc.dram_tensor("maskband", [128, 2048], bf16, kind="ExternalInput")
    id_d = nc.dram_tensor("ident", [128, 128], bf16, kind="ExternalInput")
    out_d = nc.dram_tensor("out", [S, D], bf16, kind="ExternalOutput")

    with TileContext(nc) as tc:
        with (
            tc.tile_pool(name="const", bufs=1) as cp,
            tc.tile_pool(name="persist", bufs=1) as pp,
        ):
            w_sb = cp.tile([128, 10, 640], bf16)
            wo_sb = cp.tile([128, 3, D], bf16)
            xtb = [cp.tile([128, 10, 512], bf16, name=f"xtb{jj}")
                   for jj in range(NJ)]
            rqb = [cp.tile([128, 4, 640], bf16, name=f"rqb{jj}")
                   for jj in range(NJ)]
            rkb = [cp.tile([128, 4, 320], bf16, name=f"rkb{jj}")
                   for jj in range(NJ)]
            mb_sb = cp.tile([128, 4, 512], bf16)
            id_sb = cp.tile([128, 128], bf16)
            eps_sb = cp.tile([128, 1], f32)
            c59_sb = cp.tile([128, 6], f32)
            i32 = mybir.dt.int32
            # f32 whose bits are 0x5f3759df (rsqrt bit-trick magic)
            MAGIC = 1.3211836172961055e19

            qT_sb = pp.tile([128, QH, S], bf16)
            kT_sb = pp.tile([128, KH, S], bf16)
            v_sb = pp.tile([128, NT, KH, 97], bf16)
            rbz_sb = pp.tile([96, 512], f32)

            # input DMAs, arrival-ordered: first compute needs w + xtb[0]
            for c0, c1 in ((0, 1), (1, 2), (2, 4), (4, 7), (7, 10)):
                nc.sync.dma_start(out=w_sb[:, c0:c1, :], in_=wqkv_d[:, c0:c1, :])
                nc.sync.dma_start(out=xtb[0][:, c0:c1, :],
                                  in_=xT_d[0, :, c0:c1, :])
            nc.sync.dma_start(out=rqb[0][:], in_=rq_d[:, 0:2560])
            nc.sync.dma_start(out=rkb[0][:], in_=rk_d[:, 0:1280])
            nc.sync.dma_start(out=id_sb[:], in_=id_d[:])
            # arrival order matches the phase-1 block order [0, 3, 1, 2];
            # first prefetched block halved so its first tiles aren't late
            nc.sync.dma_start(out=xtb[3][:, 0:5, :], in_=xT_d[3, :, 0:5, :])
            nc.sync.dma_start(out=xtb[3][:, 5:10, :], in_=xT_d[3, :, 5:10, :])
            for jj in (3, 1, 2):
                if jj != 3:
                    nc.sync.dma_start(out=xtb[jj][:], in_=xT_d[jj])
                nc.sync.dma_start(out=rqb[jj][:],
                                  in_=rq_d[:, 2560 * jj:2560 * (jj + 1)])
                nc.sync.dma_start(out=rkb[jj][:],
                                  in_=rk_d[:, 1280 * jj:1280 * (jj + 1)])
            nc.sync.dma_start(out=mb_sb[:], in_=mb_d[:])
            nc.sync.dma_start(out=wo_sb[:], in_=wo_d[:])

            nc.vector.memset(eps_sb[:], EPS)
            nc.vector.memset(c59_sb[:], MAGIC)
            nc.vector.memset(rbz_sb[:], 0.0)
            nc.vector.memset(v_sb[:, :, :, HD:97], 0.0)
            nc.vector.memset(v_sb[:, :, :, 96:97], 1.0)

            # ---------------- phase 1: QKV projection + norm + rope ----------
            with (
                tc.tile_pool(name="p1ps", bufs=2, space="PSUM") as ps1,
                tc.tile_pool(name="p1w", bufs=6) as wp,
            ):
                for t in [b * 4 + r for b in (0, 3, 1, 2) for r in range(4)]:
                    jj, r = t // 4, t % 4
                    qk_ps = ps1.tile([128, 480], f32, tag="qk", bufs=3)
                    v_ps = ps1.tile([128, 160], f32, tag="v", bufs=2)
                    for c in range(10):
                        lhs = xtb[jj][:, c, 128 * r:128 * (r + 1)]
                        nc.tensor.matmul(qk_ps[:], lhs, w_sb[:, c, 0:480],
                                         start=(c == 0), stop=(c == 9))
                        nc.tensor.matmul(v_ps[:], lhs, w_sb[:, c, 480:640],
                                         start=(c == 0), stop=(c == 9))
                    # v -> augmented V (ones col pre-set)
                    nc.scalar.copy(
                        v_sb[:, t, :, 0:HD],
                        v_ps.rearrange("p (g d) -> p g d", g=KH),
                    )
                    # rms stats
                    qk_nat = wp.tile([128, 480], bf16, tag="qknat")
                    nc.scalar.copy(qk_nat[:], qk_ps[:])
                    sq = wp.tile([128, 480], f32, tag="sq")
                    nc.scalar.activation(sq[:], qk_ps[:], AF.Square)
                    ssum = wp.tile([128, 6], f32, tag="ssum")
                    nc.vector.tensor_reduce(
                        ssum[:], sq.rearrange("p (h d) -> p h d", d=HD),
                        axis=mybir.AxisListType.X, op=mybir.AluOpType.add)
                    rmsq = wp.tile([128, 6], f32, tag="rmsq")
                    nc.scalar.activation(rmsq[:], ssum[:], AF.Sqrt,
                                         scale=1.0 / HD, bias=eps_sb[:])
                    rms = wp.tile([128, 6], f32, tag="rms")
                    nc.vector.reciprocal(rms[:], rmsq[:])
                    last_rmsq = rmsq

                    # rope (tables carry the norm weights)
                    qk3 = qk_nat.rearrange("p (h d) -> p h d", d=HD)
                    rope = wp.tile([128, 480], bf16, tag="rope")
                    ro3 = rope.rearrange("p (h d) -> p h d", d=HD)
                    tq1 = wp.tile([128, QH, 40], bf16, tag="tq1")
                    tq2 = wp.tile([128, QH, 40], bf16, tag="tq2")
                    tk1 = wp.tile([128, KH, 40], bf16, tag="tk1")
                    tk2 = wp.tile([128, KH, 40], bf16, tag="tk2")
                    rqt = rqb[jj][:, r, :].rearrange(
                        "p (h four d) -> p h four d", four=4, d=40)
                    rkt = rkb[jj][:, r, :].rearrange(
                        "p (h four d) -> p h four d", four=4, d=40)
                    # q halves
                    nc.vector.tensor_mul(tq1[:], qk3[:, 0:QH, 0:40], rqt[:, :, 0, :])
                    nc.vector.tensor_mul(tq2[:], qk3[:, 0:QH, 40:HD], rqt[:, :, 1, :])
                    nc.vector.tensor_sub(ro3[:, 0:QH, 0:40], tq1[:], tq2[:])
                    nc.vector.tensor_mul(tq1[:], qk3[:, 0:QH, 40:HD], rqt[:, :, 2, :])
                    nc.vector.tensor_mul(tq2[:], qk3[:, 0:QH, 0:40], rqt[:, :, 3, :])
                    nc.vector.tensor_add(ro3[:, 0:QH, 40:HD], tq1[:], tq2[:])
                    # k halves on gpsimd (frees DVE, the phase-1 bottleneck)
                    kof = QH
                    nc.gpsimd.tensor_mul(tk1[:], qk3[:, kof:kof + KH, 0:40], rkt[:, :, 0, :])
                    nc.gpsimd.tensor_mul(tk2[:], qk3[:, kof:kof + KH, 40:HD], rkt[:, :, 1, :])
                    nc.gpsimd.tensor_sub(ro3[:, kof:kof + KH, 0:40], tk1[:], tk2[:])
                    nc.gpsimd.tensor_mul(tk1[:], qk3[:, kof:kof + KH, 40:HD], rkt[:, :, 2, :])
                    nc.gpsimd.tensor_mul(tk2[:], qk3[:, kof:kof + KH, 0:40], rkt[:, :, 3, :])
                    nc.gpsimd.tensor_add(ro3[:, kof:kof + KH, 40:HD], tk1[:], tk2[:])

                    # apply rms scale per head -> qn
                    qn = wp.tile([128, 6, 128], bf16, tag="qn")
                    qn3 = qn[:, :, 0:HD]
                    for h in range(6):
                        eng = nc.vector if h < QH else nc.gpsimd
                        eng.tensor_scalar_mul(qn3[:, h, :], ro3[:, h, :],
                                              rms[:, h:h + 1])
                    # transpose the heads, two per PSUM tile -> one [80,256]
                    # drain copy per pair (split ACT/DVE)
                    for pair in range(3):
                        h0 = 2 * pair
                        tp = ps1.tile([HD, 256], bf16, tag="tp", bufs=3,
                                      name=f"tp{t}_{pair}")
                        nc.tensor.transpose(tp[:, 0:128], qn3[:, h0, :], id_sb[:])
                        nc.tensor.transpose(tp[:, 128:256], qn3[:, h0 + 1, :], id_sb[:])
                        if pair < 2:
                            dest = qT_sb[0:HD, h0:h0 + 2, 128 * t:128 * (t + 1)]
                        else:
                            dest = kT_sb[0:HD, 0:2, 128 * t:128 * (t + 1)]
                        tp2 = tp.rearrange("p (a b) -> p a b", a=2)
                        if pair % 2 == (t % 2):
                            nc.scalar.copy(dest, tp2)
                        else:
                            nc.vector.tensor_copy(dest, tp2)


                # pre-load the exp ACT table right after the LAST Sqrt
                warm = wp.tile([128, 1], f32, tag="warm")
                nc.scalar.activation(warm[:], last_rmsq[:, 0:1],
                                     AF.Exp)

            # ---------------- phase 2: attention + o_proj --------------------
            with (
                tc.tile_pool(name="ppv", bufs=2, space="PSUM") as ppv,
                tc.tile_pool(name="pop", bufs=2, space="PSUM") as pop,
                tc.tile_pool(name="psc", bufs=2, space="PSUM") as psc,
                tc.tile_pool(name="slabp", bufs=3) as slabp,
                tc.tile_pool(name="attnp", bufs=3) as attnp,
                tc.tile_pool(name="smallp", bufs=3) as smallp,
            ):
                def make_oproj_thunks(jj, atA, atB, atC):
                    """12 chunk-emitters for o_proj of q-block jj."""
                    thunks = []
                    obs = {}

                    def chunk(st, nsl_i, n0, nw):
                        def emit():
                            if st not in obs:
                                obs[st] = smallp.tile([128, D], bf16,
                                                      tag="ob", bufs=2,
                                                      name=f"ob{jj}_{st}")
                            ob = obs[st]
                            op = pop.tile([128, 512], f32, tag="op",
                                          name=f"op{jj}_{st}_{nsl_i}")
                            ops = op[:, 0:nw]
                            for ki, at in enumerate((atA, atB, atC)):
                                nc.tensor.matmul(
                                    ops, at[:, 128 * st:128 * (st + 1)],
                                    wo_sb[:, ki, n0:n0 + nw],
                                    start=(ki == 0), stop=(ki == 2))
                            if jj == NJ - 1 and nsl_i % 2 == 1:
                                nc.scalar.copy(ob[:, n0:n0 + nw], ops)
                            else:
                                nc.vector.tensor_copy(ob[:, n0:n0 + nw], ops)
                            if nsl_i == 2:
                                row0 = 512 * jj + 128 * st
                                nc.sync.dma_start(
                                    out=out_d[row0:row0 + 128, :], in_=ob[:])
                        return emit

                    for st in range(4):
                        for nsl_i, (n0, nw) in enumerate(
                                ((0, 512), (512, 512), (1024, 256))):
                            thunks.append(chunk(st, nsl_i, n0, nw))
                    return thunks

                fillers = []       # o_proj chunks of q-block j-1
                for j in range(NJ):
                    ntiles = 4 * (j + 1)
                    atA = attnp.tile([128, 512], bf16, tag="atA", name=f"atA{j}")
                    atB = attnp.tile([128, 512], bf16, tag="atB", name=f"atB{j}")
                    atC = attnp.tile([128, 512], bf16, tag="atC", name=f"atC{j}")
                    pend_pv = []
                    pend_norm = None

                    def make_pv(h, slab, j=j, ntiles=ntiles, atA=atA, atB=atB, atC=atC):
                        pv = ppv.tile([97, 512], f32, tag="pv", name=f"pv{j}_{h}")
                        thunks = []
                        for i in range(ntiles):
                            r2 = i - 4 * j
                            c0 = 128 * r2 if r2 > 0 else 0

                            def mm(i=i, c0=c0, h=h, pv=pv):
                                nc.tensor.matmul(
                                    pv[:, c0:512], v_sb[:, i, h // REP, :],
                                    slab[:, i, c0:512],
                                    start=(i == 0), stop=(i == ntiles - 1))
                            thunks.append(mm)

                        def norm(h=h, pv=pv):
                            # 1/denominator broadcast to 96 partitions via
                            # zero-padded gpsimd all-reduce
                            nc.vector.reciprocal(rbz_sb[0:1, :], pv[96:97, :])
                            rb = smallp.tile([96, 512], f32, tag="rb", bufs=3,
                                             name=f"rb{j}_{h}")
                            nc.gpsimd.partition_all_reduce(
                                rb[:], rbz_sb[:], channels=96,
                                reduce_op=bass_isa.ReduceOp.add)
                            abt = atA if h < 2 else atB
                            p0 = 64 * (h % 2)
                            nc.vector.tensor_mul(abt[p0:p0 + 64, :],
                                                 pv[0:64, :], rb[0:64, :])
                            nc.vector.tensor_mul(atC[32 * h:32 * h + 32, :],
                                                 pv[64:96, :], rb[64:96, :])
                        return thunks, norm

                    for h in range(QH):
                        g2 = h // REP
                        slab = slabp.tile([128, NT, 512], bf16, tag="slab",
                                          name=f"slab{j}_{h}")
                        for i2 in range(0, ntiles, 2):
                            glen = min(2, ntiles - i2)
                            sc = psc.tile([128, 1024], f32, tag="sc",
                                          name=f"sc{j}_{h}_{i2}")
                            for ii in range(glen):
                                i = i2 + ii
                                r2 = i - 4 * j
                                c0 = 128 * r2 if r2 > 0 else 0
                                nc.tensor.matmul(
                                    sc[:, 512 * ii + c0:512 * (ii + 1)],
                                    kT_sb[0:HD, g2, 128 * i:128 * (i + 1)],
                                    qT_sb[0:HD, h, 512 * j + c0:512 * (j + 1)],
                                    start=True, stop=True)
                            rlo = i2 - 4 * j
                            e0 = 128 * rlo if rlo > 0 else 0
                            nc.scalar.activation(
                                slab[:, i2:i2 + glen, e0:512],
                                sc.rearrange("p (a b) -> p a b",
                                             a=2)[:, 0:glen, e0:512],
                                AF.Exp, scale=SCALE)
                            # causal diagonal-block masks
                            for ii in range(glen):
                                i = i2 + ii
                                r2 = i - 4 * j
                                if 0 <= r2 < 4:
                                    nc.vector.tensor_mul(
                                        slab[:, i, 128 * r2:128 * (r2 + 1)],
                                        slab[:, i, 128 * r2:128 * (r2 + 1)],
                                        mb_sb[:, r2, 128 * r2:128 * (r2 + 1)])
                            # PE filler while ACT drains: PV of h-1, else
                            # o_proj chunks of q-block j-1
                            if pend_pv:
                                for _ in range(3):
                                    if pend_pv:
                                        pend_pv.pop(0)()
                            elif fillers:
                                fillers.pop(0)()
                        while pend_pv:
                            pend_pv.pop(0)()
                        if pend_norm is not None:
                            pend_norm()
                        pend_pv, pend_norm = make_pv(h, slab)

                    # last head's PV, interleaved with leftover fillers
                    while pend_pv:
                        pend_pv.pop(0)()
                        if pend_pv:
                            pend_pv.pop(0)()
                        if fillers:
                            fillers.pop(0)()
                    pend_norm()
                    while fillers:
                        fillers.pop(0)()
                    fillers = make_oproj_thunks(j, atA, atB, atC)

                # tail: o_proj of the last q-block
                while fillers:
                    fillers.pop(0)()
    return nc


_GRAPH_CACHE = {}


def _get_graph():
    if "nc" not in _GRAPH_CACHE:
        nc = _build_graph()
        nc.finalize()
        _GRAPH_CACHE["nc"] = nc
    return _GRAPH_CACHE["nc"]


def kernel(x, Wq, Wk, Wv, Wo, q_norm_w, k_norm_w, _trace=False):
    from concourse.bass_utils import run_bass_kernel_spmd

    x = np.asarray(x, dtype=np.float32)
    Wq = np.asarray(Wq, dtype=np.float32)
    Wk = np.asarray(Wk, dtype=np.float32)
    Wv = np.asarray(Wv, dtype=np.float32)
    Wo = np.asarray(Wo, dtype=np.float32)
    q_norm_w = np.asarray(q_norm_w, dtype=np.float32)
    k_norm_w = np.asarray(k_norm_w, dtype=np.float32)

    rq, rk, mb, ident, shards = _build_host_consts(Wq, Wk, Wv, Wo,
                                                   q_norm_w, k_norm_w)

    in_maps = []
    for core in range(NCORES):
        b = core // GROUPS
        g = core % GROUPS
        wqkv, wo = shards[g]
        xT = np.ascontiguousarray(
            x[b].T.astype(BF16).reshape(10, 128, NJ, 512).transpose(2, 1, 0, 3))
        in_maps.append({
            "xT": xT,
            "wqkv": wqkv,
            "wo": wo,
            "ropeq": rq,
            "ropek": rk,
            "maskband": mb,
            "ident": ident,
        })

    nc = _get_graph()
    res = run_bass_kernel_spmd(nc, in_maps, core_ids=list(range(NCORES)),
                               trace=_trace)
    outs = [r["out"] for r in res.results]
    full = np.zeros((B, S, D), dtype=np.float32)
    for core in range(NCORES):
        full[core // GROUPS] += np.asarray(outs[core], dtype=np.float32)
    if _trace:
        kernel.last_results = res
    return full


# revision 3
# speedup vs baseline: 1.0418x; 1.0280x over previous
"""Distributed GQA attention kernel for 8 TRN2 NeuronCores.

Problem: B=2, S=2048, D=1280, NH=16 q-heads, NKV=8 kv-heads, HD=80 (RoPE +
per-head QK RMSNorm, causal, GQA rep=2, o_proj).

Sharding: data-parallel over batch (cores 0-3 -> batch 0, cores 4-7 ->
batch 1) x tensor-parallel over kv-head groups (4 groups per batch; each
core owns 4 q heads + 2 kv heads).  Each core computes a partial o_proj
(row-shard of Wo); the host sums the 4 partials per batch (the "all-reduce")
and stacks the two batches.

Two-phase device design (per core, bf16 matmuls, f32 accumulation), as the
proven baseline, plus three scheduling fixes:
  - input DMAs split per 512-row block (first block halved) so the first
    QKV matmuls start at ~5us instead of ~12us
  - phase 2 software-pipelines PE work against the ACT exp drain: PV of the
    previous head and o_proj chunks of the previous q-block are emitted
    between score pairs, so the PE has filler work while ACT chews exp
  - output is written bf16 (halves the output DMA; host sums partials f32)
"""

import numpy as np
import ml_dtypes

B, S, D = 2, 2048, 1280
NH, NKV, HD = 16, 8, 80
REP = NH // NKV
EPS = 1e-6
THETA = 1e6
NCORES = 8
GROUPS = 4          # TP groups per batch
QH = NH // GROUPS   # 4 q heads per core
KH = NKV // GROUPS  # 2 kv heads per core
NT = S // 128       # 16 s-tiles
NJ = S // 512       # 4 q-tiles
SCALE = 1.0 / float(np.sqrt(HD))

BF16 = ml_dtypes.bfloat16


def _build_host_consts(Wq, Wk, Wv, Wo, q_norm_w, k_norm_w):
    """Per-TP-group weight shards + shared tables."""
    inv_freq = 1.0 / (THETA ** (np.arange(0, HD, 2, dtype=np.float64) / HD))
    t = np.arange(S, dtype=np.float64)
    freqs = np.outer(t, inv_freq)          # [S, 40]
    cos = np.cos(freqs).astype(np.float32)
    sin = np.sin(freqs).astype(np.float32)

    def rope_block(w):
        # [S, 160] = [C1|S1|C2|S2]; norm weight w folded in.
        c1 = cos * w[None, :40]
        s1 = sin * w[None, 40:]
        c2 = cos * w[None, 40:]
        s2 = sin * w[None, :40]
        return np.concatenate([c1, s1, c2, s2], axis=1)  # [S, 160]

    rq = np.tile(rope_block(q_norm_w), (1, QH)).astype(BF16)   # [S, 640]
    rk = np.tile(rope_block(k_norm_w), (1, KH)).astype(BF16)   # [S, 320]
    # partition-major: row p holds [t, f] blocks
    rq = np.ascontiguousarray(
        rq.reshape(NT, 128, QH * 160).transpose(1, 0, 2).reshape(128, NT * 640))
    rk = np.ascontiguousarray(
        rk.reshape(NT, 128, KH * 160).transpose(1, 0, 2).reshape(128, NT * 320))

    # band mask [128, 4, 512]: mb[p, r, c] = 1 if 128r + p <= c
    p = np.arange(128)[:, None, None]
    r = np.arange(4)[None, :, None]
    c = np.arange(512)[None, None, :]
    mb = ((128 * r + p) <= c).astype(BF16)                      # [128,4,512]
    mb = np.ascontiguousarray(mb.reshape(128, 2048))

    ident = np.eye(128, dtype=BF16)

    shards = []
    for g in range(GROUPS):
        wq = Wq[:, g * QH * HD:(g + 1) * QH * HD]               # [D, 320]
        wk = Wk[:, g * KH * HD:(g + 1) * KH * HD]               # [D, 160]
        wv = Wv[:, g * KH * HD:(g + 1) * KH * HD]               # [D, 160]
        wqkv = np.concatenate([wq, wk, wv], axis=1).astype(BF16)  # [D, 640]
        # p-major [128, 10, 640]
        wqkv = np.ascontiguousarray(
            wqkv.reshape(10, 128, 640).transpose(1, 0, 2))
        wo_g = Wo[g * QH * HD:(g + 1) * QH * HD, :]             # [320, D]
        # packed for 3-matmul o_proj: A = [h0 d0:64 | h1 d0:64],
        # B = [h2 d0:64 | h3 d0:64], C = 32-blocks [h d64:80 | zeros16]
        woA = np.concatenate([wo_g[0:64], wo_g[80:144]], axis=0)
        woB = np.concatenate([wo_g[160:224], wo_g[240:304]], axis=0)
        woC = np.zeros((128, D), np.float32)
        for h in range(QH):
            woC[32 * h:32 * h + 16] = wo_g[80 * h + 64:80 * h + 80]
        wo = np.stack([woA, woB, woC]).transpose(1, 0, 2).reshape(
            128, 3 * D).astype(BF16)                            # [128, 3*1280]
        shards.append((wqkv, wo))
    return rq, rk, mb, ident, shards


def _build_graph():
    import concourse.bacc as bacc
    import concourse.mybir as mybir
    from concourse.tile import TileContext
    from concourse import bass_isa

    f32 = mybir.dt.float32
    bf16 = mybir.dt.bfloat16
    AF = mybir.ActivationFunctionType

    nc = bacc.Bacc("TRN2", target_bir_lowering=False, debug=False,
                   num_devices=NCORES)

    xT_d = nc.dram_tensor("xT", [NJ, 128, 10, 512], bf16, kind="ExternalInput")
    wqkv_d = nc.dram_tensor("wqkv", [128, 10, 640], bf16, kind="ExternalInput")
    wo_d = nc.dram_tensor("wo", [128, 3 * D], bf16, kind="ExternalInput")
    rq_d = nc.dram_tensor("ropeq", [128, NT * 640], bf16, kind="ExternalInput")
    rk_d = nc.dram_tensor("ropek", [128, NT * 320], bf16, kind="ExternalInput")
    mb_d = nc.dram_tensor("maskband", [128, 2048], bf16, kind="ExternalInput")
    id_d = nc.dram_tensor("ident", [128, 128], bf16, kind="ExternalInput")
    out_d = nc.dram_tensor("out", [S, D], bf16, kind="ExternalOutput")

    with TileContext(nc) as tc:
        with (
            tc.tile_pool(name="const", bufs=1) as cp,
            tc.tile_pool(name="persist", bufs=1) as pp,
        ):
            w_sb = cp.tile([128, 10, 640], bf16)
            wo_sb = cp.tile([128, 3, D], bf16)
            xtb = [cp.tile([128, 10, 512], bf16, name=f"xtb{jj}")
                   for jj in range(NJ)]
            rqb = [cp.tile([128, 4, 640], bf16, name=f"rqb{jj}")
                   for jj in range(NJ)]
            rkb = [cp.tile([128, 4, 320], bf16, name=f"rkb{jj}")
                   for jj in range(NJ)]
            mb_sb = cp.tile([128, 4, 512], bf16)
            id_sb = cp.tile([128, 128], bf16)
            eps_sb = cp.tile([128, 1], f32)
            c59_sb = cp.tile([128, 6], f32)
            i32 = mybir.dt.int32
            # f32 whose bits are 0x5f3759df (rsqrt bit-trick magic)
            MAGIC = 1.3211836172961055e19

            qT_sb = pp.tile([128, QH, S], bf16)
            kT_sb = pp.tile([128, KH, S], bf16)
            v_sb = pp.tile([128, NT, KH, 97], bf16)
            rbz_sb = pp.tile([96, 512], f32)

            # input DMAs, arrival-ordered: first compute needs w + xtb[0]
            for c0, c1 in ((0, 1), (1, 2), (2, 4), (4, 7), (7, 10)):
                nc.sync.dma_start(out=w_sb[:, c0:c1, :], in_=wqkv_d[:, c0:c1, :])
                nc.sync.dma_start(out=xtb[0][:, c0:c1, :],
                                  in_=xT_d[0, :, c0:c1, :])
            nc.sync.dma_start(out=rqb[0][:], in_=rq_d[:, 0:2560])
            nc.sync.dma_start(out=rkb[0][:], in_=rk_d[:, 0:1280])
            nc.sync.dma_start(out=id_sb[:], in_=id_d[:])
            # arrival order matches the phase-1 block order [0, 3, 1, 2];
            # first prefetched block halved so its first tiles aren't late
            nc.sync.dma_start(out=xtb[3][:, 0:5, :], in_=xT_d[3, :, 0:5, :])
            nc.sync.dma_start(out=xtb[3][:, 5:10, :], in_=xT_d[3, :, 5:10, :])
            for jj in (3, 1, 2):
                if jj != 3:
                    nc.sync.dma_start(out=xtb[jj][:], in_=xT_d[jj])
                nc.sync.dma_start(out=rqb[jj][:],
                                  in_=rq_d[:, 2560 * jj:2560 * (jj + 1)])
                nc.sync.dma_start(out=rkb[jj][:],
                                  in_=rk_d[:, 1280 * jj:1280 * (jj + 1)])
            nc.sync.dma_start(out=mb_sb[:], in_=mb_d[:])
            nc.sync.dma_start(out=wo_sb[:], in_=wo_d[:])

            nc.vector.memset(eps_sb[:], EPS)
            nc.vector.memset(c59_sb[:], MAGIC)
            nc.vector.memset(rbz_sb[:], 0.0)
            nc.vector.memset(v_sb[:, :, :, HD:97], 0.0)
            nc.vector.memset(v_sb[:, :, :, 96:97], 1.0)

            # ---------------- phase 1: QKV projection + norm + rope ----------
            with (
                tc.tile_pool(name="p1ps", bufs=2, space="PSUM") as ps1,
                tc.tile_pool(name="p1w", bufs=6) as wp,
            ):
                for t in [b * 4 + r for b in (0, 3, 1, 2) for r in range(4)]:
                    jj, r = t // 4, t % 4
                    qk_ps = ps1.tile([128, 480], f32, tag="qk", bufs=3)
                    v_ps = ps1.tile([128, 160], f32, tag="v", bufs=2)
                    for c in range(10):
                        lhs = xtb[jj][:, c, 128 * r:128 * (r + 1)]
                        nc.tensor.matmul(qk_ps[:], lhs, w_sb[:, c, 0:480],
                                         start=(c == 0), stop=(c == 9))
                        nc.tensor.matmul(v_ps[:], lhs, w_sb[:, c, 480:640],
                                         start=(c == 0), stop=(c == 9))
                    # v -> augmented V (ones col pre-set)
                    nc.scalar.copy(
                        v_sb[:, t, :, 0:HD],
                        v_ps.rearrange("p (g d) -> p g d", g=KH),
                    )
                    # rms stats
                    qk_nat = wp.tile([128, 480], bf16, tag="qknat")
                    nc.scalar.copy(qk_nat[:], qk_ps[:])
                    sq = wp.tile([128, 480], f32, tag="sq")
                    nc.scalar.activation(sq[:], qk_ps[:], AF.Square)
                    ssum = wp.tile([128, 6], f32, tag="ssum")
                    nc.vector.tensor_reduce(
                        ssum[:], sq.rearrange("p (h d) -> p h d", d=HD),
                        axis=mybir.AxisListType.X, op=mybir.AluOpType.add)
                    rmsq = wp.tile([128, 6], f32, tag="rmsq")
                    nc.scalar.activation(rmsq[:], ssum[:], AF.Sqrt,
                                         scale=1.0 / HD, bias=eps_sb[:])
                    rms = wp.tile([128, 6], f32, tag="rms")
                    nc.vector.reciprocal(rms[:], rmsq[:])
                    if t == 11:
                        # last Sqrt just ran (block order 0,3,1,2): preload
                        # the exp table now so it is resident before phase 2;
                        # the remaining phase-1 ACT ops are set-agnostic Copies
                        warm = wp.tile([128, 1], f32, tag="warm")
                        nc.scalar.activation(warm[:], rmsq[:, 0:1], AF.Exp)

                    # rope (tables carry the norm weights)
                    qk3 = qk_nat.rearrange("p (h d) -> p h d", d=HD)
                    rope = wp.tile([128, 480], bf16, tag="rope")
                    ro3 = rope.rearrange("p (h d) -> p h d", d=HD)
                    tq1 = wp.tile([128, QH, 40], bf16, tag="tq1")
                    tq2 = wp.tile([128, QH, 40], bf16, tag="tq2")
                    tk1 = wp.tile([128, KH, 40], bf16, tag="tk1")
                    tk2 = wp.tile([128, KH, 40], bf16, tag="tk2")
                    rqt = rqb[jj][:, r, :].rearrange(
                        "p (h four d) -> p h four d", four=4, d=40)
                    rkt = rkb[jj][:, r, :].rearrange(
                        "p (h four d) -> p h four d", four=4, d=40)
                    # q halves
                    nc.vector.tensor_mul(tq1[:], qk3[:, 0:QH, 0:40], rqt[:, :, 0, :])
                    nc.vector.tensor_mul(tq2[:], qk3[:, 0:QH, 40:HD], rqt[:, :, 1, :])
                    nc.vector.tensor_sub(ro3[:, 0:QH, 0:40], tq1[:], tq2[:])
                    nc.vector.tensor_mul(tq1[:], qk3[:, 0:QH, 40:HD], rqt[:, :, 2, :])
                    nc.vector.tensor_mul(tq2[:], qk3[:, 0:QH, 0:40], rqt[:, :, 3, :])
                    nc.vector.tensor_add(ro3[:, 0:QH, 40:HD], tq1[:], tq2[:])
                    # k halves on gpsimd (frees DVE, the phase-1 bottleneck)
                    kof = QH
                    nc.gpsimd.tensor_mul(tk1[:], qk3[:, kof:kof + KH, 0:40], rkt[:, :, 0, :])
                    nc.gpsimd.tensor_mul(tk2[:], qk3[:, kof:kof + KH, 40:HD], rkt[:, :, 1, :])
                    nc.gpsimd.tensor_sub(ro3[:, kof:kof + KH, 0:40], tk1[:], tk2[:])
                    nc.gpsimd.tensor_mul(tk1[:], qk3[:, kof:kof + KH, 40:HD], rkt[:, :, 2, :])
                    nc.gpsimd.tensor_mul(tk2[:], qk3[:, kof:kof + KH, 0:40], rkt[:, :, 3, :])
                    nc.gpsimd.tensor_add(ro3[:, kof:kof + KH, 40:HD], tk1[:], tk2[:])

                    # apply rms scale per head -> qn
                    qn = wp.tile([128, 6, 128], bf16, tag="qn")
                    qn3 = qn[:, :, 0:HD]
                    for h in range(6):
                        eng = nc.vector if h < QH else nc.gpsimd
                        eng.tensor_scalar_mul(qn3[:, h, :], ro3[:, h, :],
                                              rms[:, h:h + 1])
                    # transpose the heads, two per PSUM tile -> one [80,256]
                    # drain copy per pair (split ACT/DVE)
                    for pair in range(3):
                        h0 = 2 * pair
                        tp = ps1.tile([HD, 256], bf16, tag="tp", bufs=3,
                                      name=f"tp{t}_{pair}")
                        nc.tensor.transpose(tp[:, 0:128], qn3[:, h0, :], id_sb[:])
                        nc.tensor.transpose(tp[:, 128:256], qn3[:, h0 + 1, :], id_sb[:])
                        if pair < 2:
                            dest = qT_sb[0:HD, h0:h0 + 2, 128 * t:128 * (t + 1)]
                        else:
                            dest = kT_sb[0:HD, 0:2, 128 * t:128 * (t + 1)]
                        tp2 = tp.rearrange("p (a b) -> p a b", a=2)
                        if pair % 2 == (t % 2):
                            nc.scalar.copy(dest, tp2)
                        else:
                            nc.vector.tensor_copy(dest, tp2)


            # ---------------- phase 2: attention + o_proj --------------------
            with (
                tc.tile_pool(name="ppv", bufs=2, space="PSUM") as ppv,
                tc.tile_pool(name="pop", bufs=2, space="PSUM") as pop,
                tc.tile_pool(name="psc", bufs=2, space="PSUM") as psc,
                tc.tile_pool(name="slabp", bufs=3) as slabp,
                tc.tile_pool(name="attnp", bufs=3) as attnp,
                tc.tile_pool(name="smallp", bufs=3) as smallp,
            ):
                def make_oproj_thunks(jj, atA, atB, atC):
                    """12 chunk-emitters for o_proj of q-block jj."""
                    thunks = []
                    obs = {}

                    def chunk(st, nsl_i, n0, nw):
                        def emit():
                            if st not in obs:
                                obs[st] = smallp.tile([128, D], bf16,
                                                      tag="ob", bufs=2,
                                                      name=f"ob{jj}_{st}")
                            ob = obs[st]
                            op = pop.tile([128, 512], f32, tag="op",
                                          name=f"op{jj}_{st}_{nsl_i}")
                            ops = op[:, 0:nw]
                            for ki, at in enumerate((atA, atB, atC)):
                                nc.tensor.matmul(
                                    ops, at[:, 128 * st:128 * (st + 1)],
                                    wo_sb[:, ki, n0:n0 + nw],
                                    start=(ki == 0), stop=(ki == 2))
                            if jj == NJ - 1 and nsl_i % 2 == 1:
                                nc.scalar.copy(ob[:, n0:n0 + nw], ops)
                            else:
                                nc.vector.tensor_copy(ob[:, n0:n0 + nw], ops)
                            row0 = 512 * jj + 128 * st
                            if jj == NJ - 1:
                                if nsl_i == 0:
                                    nc.sync.dma_start(
                                        out=out_d[row0:row0 + 128, 0:512],
                                        in_=ob[:, 0:512])
                                elif nsl_i == 2:
                                    nc.sync.dma_start(
                                        out=out_d[row0:row0 + 128, 512:D],
                                        in_=ob[:, 512:D])
                            elif nsl_i == 2:
                                nc.sync.dma_start(
                                    out=out_d[row0:row0 + 128, :], in_=ob[:])
                        return emit

                    for st in range(4):
                        for nsl_i, (n0, nw) in enumerate(
                                ((0, 512), (512, 512), (1024, 256))):
                            thunks.append(chunk(st, nsl_i, n0, nw))
                    return thunks

                fillers = []       # o_proj chunks of q-block j-1
                for j in range(NJ):
                    ntiles = 4 * (j + 1)
                    atA = attnp.tile([128, 512], bf16, tag="atA", name=f"atA{j}")
                    atB = attnp.tile([128, 512], bf16, tag="atB", name=f"atB{j}")
                    atC = attnp.tile([128, 512], bf16, tag="atC", name=f"atC{j}")
                    pend_pv = []
                    pend_norm = None

                    def make_pv(h, slab, j=j, ntiles=ntiles, atA=atA, atB=atB, atC=atC):
                        pv = ppv.tile([97, 512], f32, tag="pv", name=f"pv{j}_{h}")
                        thunks = []
                        for i in range(ntiles):
                            r2 = i - 4 * j
                            c0 = 128 * r2 if r2 > 0 else 0

                            def mm(i=i, c0=c0, h=h, pv=pv):
                                nc.tensor.matmul(
                                    pv[:, c0:512], v_sb[:, i, h // REP, :],
                                    slab[:, i, c0:512],
                                    start=(i == 0), stop=(i == ntiles - 1))
                            thunks.append(mm)

                        def norm(h=h, pv=pv):
                            abt = atA if h < 2 else atB
                            p0 = 64 * (h % 2)
                            rb = smallp.tile([96, 512], f32, tag="rb", bufs=3,
                                             name=f"rb{j}_{h}")
                            if j == NJ - 1 and h == 3:
                                # tail-critical chain: process in 128-col
                                # quarters so o_proj st-chunks unblock as
                                # their at columns complete
                                for q0 in range(0, 512, 128):
                                    qs = slice(q0, q0 + 128)
                                    nc.vector.reciprocal(rbz_sb[0:1, qs],
                                                         pv[96:97, qs])
                                    nc.gpsimd.partition_all_reduce(
                                        rb[:, qs], rbz_sb[:, qs], channels=96,
                                        reduce_op=bass_isa.ReduceOp.add)
                                    nc.vector.tensor_mul(abt[p0:p0 + 64, qs],
                                                         pv[0:64, qs],
                                                         rb[0:64, qs])
                                    nc.vector.tensor_mul(
                                        atC[32 * h:32 * h + 32, qs],
                                        pv[64:96, qs], rb[64:96, qs])
                                return
                            # 1/denominator broadcast to 96 partitions via
                            # zero-padded gpsimd all-reduce
                            nc.vector.reciprocal(rbz_sb[0:1, :], pv[96:97, :])
                            nc.gpsimd.partition_all_reduce(
                                rb[:], rbz_sb[:], channels=96,
                                reduce_op=bass_isa.ReduceOp.add)
                            nc.vector.tensor_mul(abt[p0:p0 + 64, :],
                                                 pv[0:64, :], rb[0:64, :])
                            nc.vector.tensor_mul(atC[32 * h:32 * h + 32, :],
                                                 pv[64:96, :], rb[64:96, :])
                        return thunks, norm

                    for h in range(QH):
                        g2 = h // REP
                        slab = slabp.tile([128, NT, 512], bf16, tag="slab",
                                          name=f"slab{j}_{h}")
                        for i2 in range(0, ntiles, 2):
                            glen = min(2, ntiles - i2)
                            sc = psc.tile([128, 1024], f32, tag="sc",
                                          name=f"sc{j}_{h}_{i2}")
                            for ii in range(glen):
                                i = i2 + ii
                                r2 = i - 4 * j
                                c0 = 128 * r2 if r2 > 0 else 0
                                nc.tensor.matmul(
                                    sc[:, 512 * ii + c0:512 * (ii + 1)],
                                    kT_sb[0:HD, g2, 128 * i:128 * (i + 1)],
                                    qT_sb[0:HD, h, 512 * j + c0:512 * (j + 1)],
                                    start=True, stop=True)
                            rlo = i2 - 4 * j
                            e0 = 128 * rlo if rlo > 0 else 0
                            nc.scalar.activation(
                                slab[:, i2:i2 + glen, e0:512],
                                sc.rearrange("p (a b) -> p a b",
                                             a=2)[:, 0:glen, e0:512],
                                AF.Exp, scale=SCALE)
                            # causal diagonal-block masks
                            for ii in range(glen):
                                i = i2 + ii
                                r2 = i - 4 * j
                                if 0 <= r2 < 4:
                                    nc.vector.tensor_mul(
                                        slab[:, i, 128 * r2:128 * (r2 + 1)],
                                        slab[:, i, 128 * r2:128 * (r2 + 1)],
                                        mb_sb[:, r2, 128 * r2:128 * (r2 + 1)])
                            # PE filler while ACT drains: PV of h-1, else
                            # o_proj chunks of q-block j-1
                            if pend_pv:
                                for _ in range(3):
                                    if pend_pv:
                                        pend_pv.pop(0)()
                            elif fillers:
                                fillers.pop(0)()
                        while pend_pv:
                            pend_pv.pop(0)()
                        if pend_norm is not None:
                            pend_norm()
                        pend_pv, pend_norm = make_pv(h, slab)

                    # last head's PV, interleaved with leftover fillers
                    while pend_pv:
                        pend_pv.pop(0)()
                        if pend_pv:
                            pend_pv.pop(0)()
                        if fillers:
                            fillers.pop(0)()
                    pend_norm()
                    while fillers:
                        fillers.pop(0)()
                    fillers = make_oproj_thunks(j, atA, atB, atC)

                # tail: o_proj of the last q-block
                while fillers:
                    fillers.pop(0)()
    return nc


_GRAPH_CACHE = {}


def _get_graph():
    if "nc" not in _GRAPH_CACHE:
        nc = _build_graph()
        nc.finalize()
        _GRAPH_CACHE["nc"] = nc
    return _GRAPH_CACHE["nc"]


def kernel(x, Wq, Wk, Wv, Wo, q_norm_w, k_norm_w, _trace=False):
    from concourse.bass_utils import run_bass_kernel_spmd

    x = np.asarray(x, dtype=np.float32)
    Wq = np.asarray(Wq, dtype=np.float32)
    Wk = np.asarray(Wk, dtype=np.float32)
    Wv = np.asarray(Wv, dtype=np.float32)
    Wo = np.asarray(Wo, dtype=np.float32)
    q_norm_w = np.asarray(q_norm_w, dtype=np.float32)
    k_norm_w = np.asarray(k_norm_w, dtype=np.float32)

    rq, rk, mb, ident, shards = _build_host_consts(Wq, Wk, Wv, Wo,
                                                   q_norm_w, k_norm_w)

    in_maps = []
    for core in range(NCORES):
        b = core // GROUPS
        g = core % GROUPS
        wqkv, wo = shards[g]
        xT = np.ascontiguousarray(
            x[b].T.astype(BF16).reshape(10, 128, NJ, 512).transpose(2, 1, 0, 3))
        in_maps.append({
            "xT": xT,
            "wqkv": wqkv,
            "wo": wo,
            "ropeq": rq,
            "ropek": rk,
            "maskband": mb,
            "ident": ident,
        })

    nc = _get_graph()
    res = run_bass_kernel_spmd(nc, in_maps, core_ids=list(range(NCORES)),
                               trace=_trace)
    outs = [r["out"] for r in res.results]
    full = np.zeros((B, S, D), dtype=np.float32)
    for core in range(NCORES):
        full[core // GROUPS] += np.asarray(outs[core], dtype=np.float32)
    if _trace:
        kernel.last_results = res
    return full


# revision 4
# speedup vs baseline: 1.0493x; 1.0073x over previous
"""Distributed GQA attention kernel for 8 TRN2 NeuronCores.

Problem: B=2, S=2048, D=1280, NH=16 q-heads, NKV=8 kv-heads, HD=80 (RoPE +
per-head QK RMSNorm, causal, GQA rep=2, o_proj).

Sharding: data-parallel over batch (cores 0-3 -> batch 0, cores 4-7 ->
batch 1) x tensor-parallel over kv-head groups (4 groups per batch; each
core owns 4 q heads + 2 kv heads).  Each core computes a partial o_proj
(row-shard of Wo); the host sums the 4 partials per batch (the "all-reduce")
and stacks the two batches.

Two-phase device design (per core, bf16 matmuls, f32 accumulation), as the
proven baseline, plus three scheduling fixes:
  - input DMAs split per 512-row block (first block halved) so the first
    QKV matmuls start at ~5us instead of ~12us
  - phase 2 software-pipelines PE work against the ACT exp drain: PV of the
    previous head and o_proj chunks of the previous q-block are emitted
    between score pairs, so the PE has filler work while ACT chews exp
  - output is written bf16 (halves the output DMA; host sums partials f32)
"""

import numpy as np
import ml_dtypes

B, S, D = 2, 2048, 1280
NH, NKV, HD = 16, 8, 80
REP = NH // NKV
EPS = 1e-6
THETA = 1e6
NCORES = 8
GROUPS = 4          # TP groups per batch
QH = NH // GROUPS   # 4 q heads per core
KH = NKV // GROUPS  # 2 kv heads per core
NT = S // 128       # 16 s-tiles
NJ = S // 512       # 4 q-tiles
SCALE = 1.0 / float(np.sqrt(HD))

BF16 = ml_dtypes.bfloat16


def _build_host_consts(Wq, Wk, Wv, Wo, q_norm_w, k_norm_w):
    """Per-TP-group weight shards + shared tables."""
    inv_freq = 1.0 / (THETA ** (np.arange(0, HD, 2, dtype=np.float64) / HD))
    t = np.arange(S, dtype=np.float64)
    freqs = np.outer(t, inv_freq)          # [S, 40]
    cos = np.cos(freqs).astype(np.float32)
    sin = np.sin(freqs).astype(np.float32)

    def rope_block(w):
        # [S, 160] = [C1|S1|C2|S2]; norm weight w folded in.
        c1 = cos * w[None, :40]
        s1 = sin * w[None, 40:]
        c2 = cos * w[None, 40:]
        s2 = sin * w[None, :40]
        return np.concatenate([c1, s1, c2, s2], axis=1)  # [S, 160]

    rq = np.tile(rope_block(q_norm_w), (1, QH)).astype(BF16)   # [S, 640]
    rk = np.tile(rope_block(k_norm_w), (1, KH)).astype(BF16)   # [S, 320]
    # partition-major: row p holds [t, f] blocks
    rq = np.ascontiguousarray(
        rq.reshape(NT, 128, QH * 160).transpose(1, 0, 2).reshape(128, NT * 640))
    rk = np.ascontiguousarray(
        rk.reshape(NT, 128, KH * 160).transpose(1, 0, 2).reshape(128, NT * 320))

    # band mask [128, 4, 512]: mb[p, r, c] = 1 if 128r + p <= c
    p = np.arange(128)[:, None, None]
    r = np.arange(4)[None, :, None]
    c = np.arange(512)[None, None, :]
    mb = ((128 * r + p) <= c).astype(BF16)                      # [128,4,512]
    mb = np.ascontiguousarray(mb.reshape(128, 2048))

    ident = np.eye(128, dtype=BF16)

    shards = []
    for g in range(GROUPS):
        wq = Wq[:, g * QH * HD:(g + 1) * QH * HD]               # [D, 320]
        wk = Wk[:, g * KH * HD:(g + 1) * KH * HD]               # [D, 160]
        wv = Wv[:, g * KH * HD:(g + 1) * KH * HD]               # [D, 160]
        wqkv = np.concatenate([wq, wk, wv], axis=1).astype(BF16)  # [D, 640]
        # p-major [128, 10, 640]
        wqkv = np.ascontiguousarray(
            wqkv.reshape(10, 128, 640).transpose(1, 0, 2))
        wo_g = Wo[g * QH * HD:(g + 1) * QH * HD, :]             # [320, D]
        # packed for 3-matmul o_proj: A = [h0 d0:64 | h1 d0:64],
        # B = [h2 d0:64 | h3 d0:64], C = 32-blocks [h d64:80 | zeros16]
        woA = np.concatenate([wo_g[0:64], wo_g[80:144]], axis=0)
        woB = np.concatenate([wo_g[160:224], wo_g[240:304]], axis=0)
        woC = np.zeros((128, D), np.float32)
        for h in range(QH):
            woC[32 * h:32 * h + 16] = wo_g[80 * h + 64:80 * h + 80]
        wo = np.stack([woA, woB, woC]).transpose(1, 0, 2).reshape(
            128, 3 * D).astype(BF16)                            # [128, 3*1280]
        shards.append((wqkv, wo))
    return rq, rk, mb, ident, shards


def _build_graph():
    import concourse.bacc as bacc
    import concourse.mybir as mybir
    from concourse.tile import TileContext
    from concourse import bass_isa

    f32 = mybir.dt.float32
    bf16 = mybir.dt.bfloat16
    AF = mybir.ActivationFunctionType

    nc = bacc.Bacc("TRN2", target_bir_lowering=False, debug=False,
                   num_devices=NCORES)

    xT_d = nc.dram_tensor("xT", [NJ, 128, 10, 512], bf16, kind="ExternalInput")
    wqkv_d = nc.dram_tensor("wqkv", [128, 10, 640], bf16, kind="ExternalInput")
    wo_d = nc.dram_tensor("wo", [128, 3 * D], bf16, kind="ExternalInput")
    rq_d = nc.dram_tensor("ropeq", [128, NT * 640], bf16, kind="ExternalInput")
    rk_d = nc.dram_tensor("ropek", [128, NT * 320], bf16, kind="ExternalInput")
    mb_d = nc.dram_tensor("maskband", [128, 2048], bf16, kind="ExternalInput")
    id_d = nc.dram_tensor("ident", [128, 128], bf16, kind="ExternalInput")
    out_d = nc.dram_tensor("out", [S, D], bf16, kind="ExternalOutput")

    with TileContext(nc) as tc:
        with (
            tc.tile_pool(name="const", bufs=1) as cp,
            tc.tile_pool(name="persist", bufs=1) as pp,
        ):
            w_sb = cp.tile([128, 10, 640], bf16)
            wo_sb = cp.tile([128, 3, D], bf16)
            xtb = [cp.tile([128, 10, 512], bf16, name=f"xtb{jj}")
                   for jj in range(NJ)]
            rqb = [cp.tile([128, 4, 640], bf16, name=f"rqb{jj}")
                   for jj in range(NJ)]
            rkb = [cp.tile([128, 4, 320], bf16, name=f"rkb{jj}")
                   for jj in range(NJ)]
            mb_sb = cp.tile([128, 4, 512], bf16)
            id_sb = cp.tile([128, 128], bf16)
            eps_sb = cp.tile([128, 1], f32)
            c59_sb = cp.tile([128, 6], f32)
            i32 = mybir.dt.int32
            # f32 whose bits are 0x5f3759df (rsqrt bit-trick magic)
            MAGIC = 1.3211836172961055e19

            qT_sb = pp.tile([128, QH, S], bf16)
            kT_sb = pp.tile([128, KH, S], bf16)
            v_sb = pp.tile([128, NT, KH, 97], bf16)
            rbz_sb = pp.tile([96, 512], f32)

            # input DMAs, arrival-ordered: first compute needs w + xtb[0]
            for c0, c1 in ((0, 1), (1, 2), (2, 4), (4, 7), (7, 10)):
                nc.sync.dma_start(out=w_sb[:, c0:c1, :], in_=wqkv_d[:, c0:c1, :])
                nc.sync.dma_start(out=xtb[0][:, c0:c1, :],
                                  in_=xT_d[0, :, c0:c1, :])
            nc.sync.dma_start(out=rqb[0][:], in_=rq_d[:, 0:2560])
            nc.sync.dma_start(out=rkb[0][:], in_=rk_d[:, 0:1280])
            nc.sync.dma_start(out=id_sb[:], in_=id_d[:])
            # arrival order matches the phase-1 block order [0, 3, 1, 2];
            # first prefetched block halved so its first tiles aren't late
            nc.sync.dma_start(out=xtb[3][:, 0:5, :], in_=xT_d[3, :, 0:5, :])
            nc.sync.dma_start(out=xtb[3][:, 5:10, :], in_=xT_d[3, :, 5:10, :])
            for jj in (3, 1, 2):
                if jj != 3:
                    nc.sync.dma_start(out=xtb[jj][:], in_=xT_d[jj])
                nc.sync.dma_start(out=rqb[jj][:],
                                  in_=rq_d[:, 2560 * jj:2560 * (jj + 1)])
                nc.sync.dma_start(out=rkb[jj][:],
                                  in_=rk_d[:, 1280 * jj:1280 * (jj + 1)])
            nc.sync.dma_start(out=mb_sb[:], in_=mb_d[:])
            nc.sync.dma_start(out=wo_sb[:], in_=wo_d[:])

            nc.vector.memset(eps_sb[:], EPS)
            nc.vector.memset(c59_sb[:], MAGIC)
            nc.vector.memset(rbz_sb[:], 0.0)
            nc.vector.memset(v_sb[:, :, :, HD:97], 0.0)
            nc.vector.memset(v_sb[:, :, :, 96:97], 1.0)

            # ---------------- phase 1: QKV projection + norm + rope ----------
            with (
                tc.tile_pool(name="p1ps", bufs=2, space="PSUM") as ps1,
                tc.tile_pool(name="p1w", bufs=6) as wp,
            ):
                for t in [b * 4 + r for b in (0, 3, 1, 2) for r in range(4)]:
                    jj, r = t // 4, t % 4
                    qk_ps = ps1.tile([128, 480], f32, tag="qk", bufs=3)
                    v_ps = ps1.tile([128, 160], f32, tag="v", bufs=2)
                    for c in range(10):
                        lhs = xtb[jj][:, c, 128 * r:128 * (r + 1)]
                        nc.tensor.matmul(qk_ps[:], lhs, w_sb[:, c, 0:480],
                                         start=(c == 0), stop=(c == 9))
                        nc.tensor.matmul(v_ps[:], lhs, w_sb[:, c, 480:640],
                                         start=(c == 0), stop=(c == 9))
                    # v -> augmented V (ones col pre-set)
                    nc.scalar.copy(
                        v_sb[:, t, :, 0:HD],
                        v_ps.rearrange("p (g d) -> p g d", g=KH),
                    )
                    # rms stats
                    qk_nat = wp.tile([128, 480], bf16, tag="qknat")
                    nc.scalar.copy(qk_nat[:], qk_ps[:])
                    sq = wp.tile([128, 480], f32, tag="sq")
                    nc.scalar.activation(sq[:], qk_ps[:], AF.Square)
                    ssum = wp.tile([128, 6], f32, tag="ssum")
                    nc.vector.tensor_reduce(
                        ssum[:], sq.rearrange("p (h d) -> p h d", d=HD),
                        axis=mybir.AxisListType.X, op=mybir.AluOpType.add)
                    rmsq = wp.tile([128, 6], f32, tag="rmsq")
                    nc.scalar.activation(rmsq[:], ssum[:], AF.Sqrt,
                                         scale=1.0 / HD, bias=eps_sb[:])
                    rms = wp.tile([128, 6], f32, tag="rms")
                    nc.vector.reciprocal(rms[:], rmsq[:])
                    if t == 11:
                        # last Sqrt just ran (block order 0,3,1,2): preload
                        # the exp table now so it is resident before phase 2;
                        # the remaining phase-1 ACT ops are set-agnostic Copies
                        warm = wp.tile([128, 1], f32, tag="warm")
                        nc.scalar.activation(warm[:], rmsq[:, 0:1], AF.Exp)

                    # rope (tables carry the norm weights)
                    qk3 = qk_nat.rearrange("p (h d) -> p h d", d=HD)
                    rope = wp.tile([128, 480], bf16, tag="rope")
                    ro3 = rope.rearrange("p (h d) -> p h d", d=HD)
                    tq1 = wp.tile([128, QH, 40], bf16, tag="tq1")
                    tq2 = wp.tile([128, QH, 40], bf16, tag="tq2")
                    tk1 = wp.tile([128, KH, 40], bf16, tag="tk1")
                    tk2 = wp.tile([128, KH, 40], bf16, tag="tk2")
                    rqt = rqb[jj][:, r, :].rearrange(
                        "p (h four d) -> p h four d", four=4, d=40)
                    rkt = rkb[jj][:, r, :].rearrange(
                        "p (h four d) -> p h four d", four=4, d=40)
                    # q halves
                    nc.vector.tensor_mul(tq1[:], qk3[:, 0:QH, 0:40], rqt[:, :, 0, :])
                    nc.vector.tensor_mul(tq2[:], qk3[:, 0:QH, 40:HD], rqt[:, :, 1, :])
                    nc.vector.tensor_sub(ro3[:, 0:QH, 0:40], tq1[:], tq2[:])
                    nc.vector.tensor_mul(tq1[:], qk3[:, 0:QH, 40:HD], rqt[:, :, 2, :])
                    nc.vector.tensor_mul(tq2[:], qk3[:, 0:QH, 0:40], rqt[:, :, 3, :])
                    nc.vector.tensor_add(ro3[:, 0:QH, 40:HD], tq1[:], tq2[:])
                    # k halves on gpsimd (frees DVE, the phase-1 bottleneck)
                    kof = QH
                    nc.gpsimd.tensor_mul(tk1[:], qk3[:, kof:kof + KH, 0:40], rkt[:, :, 0, :])
                    nc.gpsimd.tensor_mul(tk2[:], qk3[:, kof:kof + KH, 40:HD], rkt[:, :, 1, :])
                    nc.gpsimd.tensor_sub(ro3[:, kof:kof + KH, 0:40], tk1[:], tk2[:])
                    nc.gpsimd.tensor_mul(tk1[:], qk3[:, kof:kof + KH, 40:HD], rkt[:, :, 2, :])
                    nc.gpsimd.tensor_mul(tk2[:], qk3[:, kof:kof + KH, 0:40], rkt[:, :, 3, :])
                    nc.gpsimd.tensor_add(ro3[:, kof:kof + KH, 40:HD], tk1[:], tk2[:])

                    # apply rms scale per head -> qn
                    qn = wp.tile([128, 6, 128], bf16, tag="qn")
                    qn3 = qn[:, :, 0:HD]
                    for h in range(6):
                        nc.vector.tensor_scalar_mul(qn3[:, h, :], ro3[:, h, :],
                                                    rms[:, h:h + 1])
                    # transpose the heads, two per PSUM tile -> one [80,256]
                    # drain copy per pair (split ACT/DVE)
                    for pair in range(3):
                        h0 = 2 * pair
                        tp = ps1.tile([HD, 256], bf16, tag="tp", bufs=3,
                                      name=f"tp{t}_{pair}")
                        nc.tensor.transpose(tp[:, 0:128], qn3[:, h0, :], id_sb[:])
                        nc.tensor.transpose(tp[:, 128:256], qn3[:, h0 + 1, :], id_sb[:])
                        if pair < 2:
                            dest = qT_sb[0:HD, h0:h0 + 2, 128 * t:128 * (t + 1)]
                        else:
                            dest = kT_sb[0:HD, 0:2, 128 * t:128 * (t + 1)]
                        tp2 = tp.rearrange("p (a b) -> p a b", a=2)
                        if pair % 2 == (t % 2):
                            nc.scalar.copy(dest, tp2)
                        else:
                            nc.vector.tensor_copy(dest, tp2)


            # ---------------- phase 2: attention + o_proj --------------------
            with (
                tc.tile_pool(name="ppv", bufs=2, space="PSUM") as ppv,
                tc.tile_pool(name="pop", bufs=2, space="PSUM") as pop,
                tc.tile_pool(name="psc", bufs=2, space="PSUM") as psc,
                tc.tile_pool(name="slabp", bufs=3) as slabp,
                tc.tile_pool(name="attnp", bufs=3) as attnp,
                tc.tile_pool(name="smallp", bufs=3) as smallp,
            ):
                def make_oproj_thunks(jj, atA, atB, atC):
                    """12 chunk-emitters for o_proj of q-block jj."""
                    thunks = []
                    obs = {}

                    def chunk(st, nsl_i, n0, nw):
                        def emit():
                            if st not in obs:
                                obs[st] = smallp.tile([128, D], bf16,
                                                      tag="ob", bufs=2,
                                                      name=f"ob{jj}_{st}")
                            ob = obs[st]
                            op = pop.tile([128, 512], f32, tag="op",
                                          name=f"op{jj}_{st}_{nsl_i}")
                            ops = op[:, 0:nw]
                            for ki, at in enumerate((atA, atB, atC)):
                                nc.tensor.matmul(
                                    ops, at[:, 128 * st:128 * (st + 1)],
                                    wo_sb[:, ki, n0:n0 + nw],
                                    start=(ki == 0), stop=(ki == 2))
                            if jj == NJ - 1 and nsl_i % 2 == 1:
                                nc.scalar.copy(ob[:, n0:n0 + nw], ops)
                            else:
                                nc.vector.tensor_copy(ob[:, n0:n0 + nw], ops)
                            row0 = 512 * jj + 128 * st
                            if jj == NJ - 1:
                                if nsl_i == 0:
                                    nc.sync.dma_start(
                                        out=out_d[row0:row0 + 128, 0:512],
                                        in_=ob[:, 0:512])
                                elif nsl_i == 2:
                                    nc.sync.dma_start(
                                        out=out_d[row0:row0 + 128, 512:D],
                                        in_=ob[:, 512:D])
                            elif nsl_i == 2:
                                nc.sync.dma_start(
                                    out=out_d[row0:row0 + 128, :], in_=ob[:])
                        return emit

                    for st in range(4):
                        for nsl_i, (n0, nw) in enumerate(
                                ((0, 512), (512, 512), (1024, 256))):
                            thunks.append(chunk(st, nsl_i, n0, nw))
                    return thunks

                fillers = []       # o_proj chunks of q-block j-1
                for j in range(NJ):
                    ntiles = 4 * (j + 1)
                    atA = attnp.tile([128, 512], bf16, tag="atA", name=f"atA{j}")
                    atB = attnp.tile([128, 512], bf16, tag="atB", name=f"atB{j}")
                    atC = attnp.tile([128, 512], bf16, tag="atC", name=f"atC{j}")
                    pend_pv = []
                    pend_norm = None

                    def make_pv(h, slab, j=j, ntiles=ntiles, atA=atA, atB=atB, atC=atC):
                        pv = ppv.tile([97, 512], f32, tag="pv", name=f"pv{j}_{h}")
                        thunks = []
                        for i in range(ntiles):
                            r2 = i - 4 * j
                            c0 = 128 * r2 if r2 > 0 else 0

                            def mm(i=i, c0=c0, h=h, pv=pv):
                                nc.tensor.matmul(
                                    pv[:, c0:512], v_sb[:, i, h // REP, :],
                                    slab[:, i, c0:512],
                                    start=(i == 0), stop=(i == ntiles - 1))
                            thunks.append(mm)

                        def norm(h=h, pv=pv):
                            abt = atA if h < 2 else atB
                            p0 = 64 * (h % 2)
                            rb = smallp.tile([96, 512], f32, tag="rb", bufs=3,
                                             name=f"rb{j}_{h}")
                            if j == NJ - 1 and h == 3:
                                # tail-critical chain: process in 128-col
                                # quarters so o_proj st-chunks unblock as
                                # their at columns complete
                                for q0 in range(0, 512, 128):
                                    qs = slice(q0, q0 + 128)
                                    nc.vector.reciprocal(rbz_sb[0:1, qs],
                                                         pv[96:97, qs])
                                    nc.gpsimd.partition_all_reduce(
                                        rb[:, qs], rbz_sb[:, qs], channels=96,
                                        reduce_op=bass_isa.ReduceOp.add)
                                    nc.vector.tensor_mul(abt[p0:p0 + 64, qs],
                                                         pv[0:64, qs],
                                                         rb[0:64, qs])
                                    nc.vector.tensor_mul(
                                        atC[32 * h:32 * h + 32, qs],
                                        pv[64:96, qs], rb[64:96, qs])
                                return
                            # 1/denominator broadcast to 96 partitions via
                            # zero-padded gpsimd all-reduce
                            nc.vector.reciprocal(rbz_sb[0:1, :], pv[96:97, :])
                            nc.gpsimd.partition_all_reduce(
                                rb[:], rbz_sb[:], channels=96,
                                reduce_op=bass_isa.ReduceOp.add)
                            nc.vector.tensor_mul(abt[p0:p0 + 64, :],
                                                 pv[0:64, :], rb[0:64, :])
                            nc.vector.tensor_mul(atC[32 * h:32 * h + 32, :],
                                                 pv[64:96, :], rb[64:96, :])
                        return thunks, norm

                    for h in range(QH):
                        g2 = h // REP
                        slab = slabp.tile([128, NT, 512], bf16, tag="slab",
                                          name=f"slab{j}_{h}")
                        for i2 in range(0, ntiles, 2):
                            glen = min(2, ntiles - i2)
                            sc = psc.tile([128, 1024], f32, tag="sc",
                                          name=f"sc{j}_{h}_{i2}")
                            for ii in range(glen):
                                i = i2 + ii
                                r2 = i - 4 * j
                                c0 = 128 * r2 if r2 > 0 else 0
                                nc.tensor.matmul(
                                    sc[:, 512 * ii + c0:512 * (ii + 1)],
                                    kT_sb[0:HD, g2, 128 * i:128 * (i + 1)],
                                    qT_sb[0:HD, h, 512 * j + c0:512 * (j + 1)],
                                    start=True, stop=True)
                            rlo = i2 - 4 * j
                            e0 = 128 * rlo if rlo > 0 else 0
                            nc.scalar.activation(
                                slab[:, i2:i2 + glen, e0:512],
                                sc.rearrange("p (a b) -> p a b",
                                             a=2)[:, 0:glen, e0:512],
                                AF.Exp, scale=SCALE)
                            # causal diagonal-block masks
                            for ii in range(glen):
                                i = i2 + ii
                                r2 = i - 4 * j
                                if 0 <= r2 < 4:
                                    nc.vector.tensor_mul(
                                        slab[:, i, 128 * r2:128 * (r2 + 1)],
                                        slab[:, i, 128 * r2:128 * (r2 + 1)],
                                        mb_sb[:, r2, 128 * r2:128 * (r2 + 1)])
                            # PE filler while ACT drains: PV of h-1, else
                            # o_proj chunks of q-block j-1
                            if pend_pv:
                                for _ in range(3):
                                    if pend_pv:
                                        pend_pv.pop(0)()
                            elif fillers:
                                fillers.pop(0)()
                        while pend_pv:
                            pend_pv.pop(0)()
                        if pend_norm is not None:
                            pend_norm()
                        pend_pv, pend_norm = make_pv(h, slab)

                    # last head's PV, interleaved with leftover fillers
                    while pend_pv:
                        pend_pv.pop(0)()
                        if pend_pv:
                            pend_pv.pop(0)()
                        if fillers:
                            fillers.pop(0)()
                    pend_norm()
                    while fillers:
                        fillers.pop(0)()
                    fillers = make_oproj_thunks(j, atA, atB, atC)

                # tail: o_proj of the last q-block
                while fillers:
                    fillers.pop(0)()
    return nc


_GRAPH_CACHE = {}


def _get_graph():
    if "nc" not in _GRAPH_CACHE:
        nc = _build_graph()
        nc.finalize()
        _GRAPH_CACHE["nc"] = nc
    return _GRAPH_CACHE["nc"]


def kernel(x, Wq, Wk, Wv, Wo, q_norm_w, k_norm_w, _trace=False):
    from concourse.bass_utils import run_bass_kernel_spmd

    x = np.asarray(x, dtype=np.float32)
    Wq = np.asarray(Wq, dtype=np.float32)
    Wk = np.asarray(Wk, dtype=np.float32)
    Wv = np.asarray(Wv, dtype=np.float32)
    Wo = np.asarray(Wo, dtype=np.float32)
    q_norm_w = np.asarray(q_norm_w, dtype=np.float32)
    k_norm_w = np.asarray(k_norm_w, dtype=np.float32)

    rq, rk, mb, ident, shards = _build_host_consts(Wq, Wk, Wv, Wo,
                                                   q_norm_w, k_norm_w)

    in_maps = []
    for core in range(NCORES):
        b = core // GROUPS
        g = core % GROUPS
        wqkv, wo = shards[g]
        xT = np.ascontiguousarray(
            x[b].T.astype(BF16).reshape(10, 128, NJ, 512).transpose(2, 1, 0, 3))
        in_maps.append({
            "xT": xT,
            "wqkv": wqkv,
            "wo": wo,
            "ropeq": rq,
            "ropek": rk,
            "maskband": mb,
            "ident": ident,
        })

    nc = _get_graph()
    res = run_bass_kernel_spmd(nc, in_maps, core_ids=list(range(NCORES)),
                               trace=_trace)
    outs = [r["out"] for r in res.results]
    full = np.zeros((B, S, D), dtype=np.float32)
    for core in range(NCORES):
        full[core // GROUPS] += np.asarray(outs[core], dtype=np.float32)
    if _trace:
        kernel.last_results = res
    return full
